# revision 12
# baseline (speedup 1.0000x reference)
"""BiMamba Trainium2 kernel — 8-core SPMD, v2.

Sharding: core = b*4 + dir*2 + nh  (b: batch, dir: fwd/rev, nh: state half).
Each core runs the full mamba pipeline for its (b, dir) on all 768 inner
channels but only its 8 of 16 SSM states, pushes its partial through its
direction's half of the final 1x1 conv, then per-chunk ReduceScatter(add)
over its batch group of 4 cores sums partials and hands each core 192
(permuted) channels = 96 GLU `a` channels + their 96 `b` partners.
GLU + GroupNorm finish locally (tiny AllReduce for the GN stats).

v2: T=1024 time chunks; depthwise conv = xi in-projection + 4 diagonal
tap matmuls accumulated in PSUM; silu/exp/ln on Scalar straight from
PSUM; scan carry injected via a leading scan column; u=dx*B on GpSimd;
q=h*C on Vector; D-skip folded into the state-reduce as a diagonal
matmul; per-chunk ReduceScatter overlapped with later chunks.
"""
import numpy as np
import ml_dtypes

import concourse.bass as bass
import concourse.bacc as bacc_mod
import concourse.mybir as mybir
import concourse.tile as tile
from concourse.tile_rust import add_dep_helper
from concourse.bass_utils import run_bass_kernel_spmd

F32 = mybir.dt.float32
BF16 = mybir.dt.bfloat16
AF = mybir.ActivationFunctionType
OP = mybir.AluOpType

D_MODEL = 384
D_INNER = 768
D_STATE = 16
D_CONV = 4
DT_RANK = 24
B = 2
L = 4096
T = 1024                # time chunk
TP = T + 2              # carry col + pass-through col (4B-aligned data)
NCH = L // T
NH = 8                  # states per core
NM = 40                 # xproj out rows: 24 dt + 8 B + 8 C
RG = [[0, 1, 2, 3], [4, 5, 6, 7]]   # batch groups
GN_N = float(D_MODEL * L)

bf = ml_dtypes.bfloat16


def build_program():
    nc = bacc_mod.Bacc(num_devices=8)

    x_in = nc.dram_tensor("x_bc", [128, 3, L], BF16, kind="ExternalInput")
    w_zg = nc.dram_tensor("w_zg", [128, 3, D_INNER], BF16, kind="ExternalInput")
    w_xi = nc.dram_tensor("w_xi", [128, 3, D_INNER], BF16, kind="ExternalInput")
    w_tap = nc.dram_tensor("w_tap", [128, 6 * D_CONV, 128], BF16, kind="ExternalInput")
    w_xp = nc.dram_tensor("w_xp", [128, 6, NM], BF16, kind="ExternalInput")
    w_dt = nc.dram_tensor("w_dt", [DT_RANK, D_INNER], BF16, kind="ExternalInput")
    w_out = nc.dram_tensor("w_out", [128, 6, D_MODEL], BF16, kind="ExternalInput")
    w_zc = nc.dram_tensor("w_zc", [128, 3, 2 * D_MODEL], BF16, kind="ExternalInput")
    w_dsk = nc.dram_tensor("w_dsk", [128, 6, 128], BF16, kind="ExternalInput")
    conv_b = nc.dram_tensor("conv_b", [128, 6], F32, kind="ExternalInput")
    dt_b = nc.dram_tensor("dt_b", [128, 6], F32, kind="ExternalInput")
    a_sc = nc.dram_tensor("a_sc", [128, 6, NH], F32, kind="ExternalInput")
    cb_a = nc.dram_tensor("cb_a", [96, 1], F32, kind="ExternalInput")
    cb_b = nc.dram_tensor("cb_b", [96, 1], F32, kind="ExternalInput")
    gnw = nc.dram_tensor("gnw", [96, 1], F32, kind="ExternalInput")
    gnb = nc.dram_tensor("gnb", [96, 1], F32, kind="ExternalInput")
    y_out = nc.dram_tensor("y_out", [96, L], F32, kind="ExternalOutput")

    bc_stage = nc.dram_tensor("bc_stage", [2 * NH, L], BF16)
    z_parts = [nc.dram_tensor(f"z_part{c}", [D_INNER, T], BF16) for c in range(NCH)]
    z_reds = [nc.dram_tensor(f"z_red{c}", [192, T], BF16) for c in range(NCH)]
    glu_dram = nc.dram_tensor("glu_dram", [96, L], F32)
    gn_in = nc.dram_tensor("gn_in", [1, 2], F32)
    gn_out = nc.dram_tensor("gn_out", [1, 2], F32)
    mr_dram = nc.dram_tensor("mr_dram", [1, 2], F32)

    ident_dram = nc.inline_tensor(np.eye(128, dtype=bf), name="ident")

    with tile.TileContext(nc) as tc:
        _body(tc, nc, x_in, w_zg, w_xi, w_tap, w_xp, w_dt, w_out, w_zc, w_dsk,
              conv_b, dt_b, a_sc, cb_a, cb_b, gnw, gnb, y_out,
              bc_stage, z_parts, z_reds, glu_dram, gn_in, gn_out, mr_dram,
              ident_dram)
    if not nc.is_finalized():
        nc.finalize()
    return nc


def _body(tc, nc, x_in, w_zg, w_xi, w_tap, w_xp, w_dt, w_out, w_zc, w_dsk,
          conv_b, dt_b, a_sc, cb_a, cb_b, gnw, gnb, y_out,
          bc_stage, z_parts, z_reds, glu_dram, gn_in, gn_out, mr_dram,
          ident_dram):
    from contextlib import ExitStack

    with ExitStack() as ctx:
        singles = ctx.enter_context(tc.tile_pool(name="singles", bufs=1))
        sb_wzg = singles.tile([128, 3, D_INNER], BF16)
        nc.sync.dma_start(out=sb_wzg, in_=w_zg[:])
        sb_wxi = singles.tile([128, 3, D_INNER], BF16)
        nc.sync.dma_start(out=sb_wxi, in_=w_xi[:])
        sb_wxp = singles.tile([128, 6, NM], BF16)
        nc.sync.dma_start(out=sb_wxp, in_=w_xp[:])
        sb_wdt = singles.tile([DT_RANK, D_INNER], BF16)
        nc.sync.dma_start(out=sb_wdt, in_=w_dt[:])
        sb_wout = singles.tile([128, 6, D_MODEL], BF16)
        nc.sync.dma_start(out=sb_wout, in_=w_out[:])
        sb_wzc = singles.tile([128, 3, 2 * D_MODEL], BF16)
        nc.sync.dma_start(out=sb_wzc, in_=w_zc[:])
        sb_wdsk = singles.tile([128, 6, 128], BF16)
        nc.sync.dma_start(out=sb_wdsk, in_=w_dsk[:])
        sb_cb = singles.tile([128, 6], F32)
        nc.sync.dma_start(out=sb_cb, in_=conv_b[:])
        sb_dtb = singles.tile([128, 6], F32)
        nc.sync.dma_start(out=sb_dtb, in_=dt_b[:])
        sb_asc = singles.tile([128, 6, NH], F32)
        nc.sync.dma_start(out=sb_asc, in_=a_sc[:])
        sb_id = singles.tile([128, 128], BF16)
        nc.sync.dma_start(out=sb_id, in_=ident_dram[:])
        sb_carry = singles.tile([128, 6, NH], F32)
        nc.vector.memset(sb_carry, 0.0)

        pA = ctx.enter_context(tc.tile_pool(name="pA", bufs=2, space="PSUM"))
        pPY = ctx.enter_context(tc.tile_pool(name="pPY", bufs=2, space="PSUM"))

        p_x = ctx.enter_context(tc.tile_pool(name="p_x", bufs=1))
        p_sz = ctx.enter_context(tc.tile_pool(name="p_sz", bufs=2))
        p_xi = ctx.enter_context(tc.tile_pool(name="p_xi", bufs=1))
        p_xc = ctx.enter_context(tc.tile_pool(name="p_xc", bufs=1))
        p_xdbl = ctx.enter_context(tc.tile_pool(name="p_xdbl", bufs=1))
        p_dl = ctx.enter_context(tc.tile_pool(name="p_dl", bufs=1))
        p_bc = ctx.enter_context(tc.tile_pool(name="p_bc", bufs=1))
        p_scan = ctx.enter_context(tc.tile_pool(name="p_scan", bufs=1))

        def mid_bcast(ap2d, reps):
            return bass.AP(tensor=ap2d.tensor, offset=ap2d.offset,
                           ap=[ap2d.ap[0], [0, reps], ap2d.ap[1]])

        # xi rolling tile (halo in cols 0:3, persistent across chunks)
        sb_xi = p_xi.tile([128, 6, T + 3], BF16)
        nc.gpsimd.memset(sb_xi[:, :, 0:3], 0.0)

        # double-buffered scan tiles; col 0 of da stays 0 forever
        sb_das = [p_scan.tile([128, NH, TP], BF16, name=f"sb_da{i}")
                  for i in range(2)]
        sb_us = [p_scan.tile([128, NH, TP], BF16, name=f"sb_u{i}")
                 for i in range(2)]
        sb_h = p_scan.tile([128, NH, TP], BF16)
        sb_dx = p_scan.tile([128, T], BF16)
        for t_ in sb_das:
            nc.gpsimd.memset(t_[:, :, 0:1], 0.0)
            nc.gpsimd.memset(t_[:, :, 1:2], 1.0)
        for t_ in sb_us:
            nc.gpsimd.memset(t_[:, :, 1:2], 0.0)

        last_act = [None]

        def act(out, in_, func, chain=True, **kw):
            i = nc.scalar.activation(out=out, in_=in_, func=func, **kw)
            if chain and last_act[0] is not None:
                add_dep_helper(i.ins, last_act[0].ins, sync=False,
                               reason="ACT table grouping")
            if chain:
                last_act[0] = i
            return i

        for c in range(NCH):
            sl = slice(c * T, (c + 1) * T)
            last_act[0] = None   # allow cross-chunk S overlap

            sb_x = p_x.tile([128, 3, T], BF16, tag="x")
            nc.sync.dma_start(out=sb_x, in_=x_in[:, :, sl])

            # ---- z-gate -> silu -> sz ----
            sb_sz = p_sz.tile([128, 6, T], BF16, tag="sz")
            for mt in range(6):
                ps = pA.tile([128, T], F32, tag="mm")
                for fh in range(2):
                    for kt in range(3):
                        nc.tensor.matmul(
                            ps[:, fh * 512:(fh + 1) * 512],
                            sb_wzg[:, kt, mt * 128:(mt + 1) * 128],
                            sb_x[:, kt, fh * 512:(fh + 1) * 512],
                            start=(kt == 0), stop=(kt == 2))
                act(out=sb_sz[:, mt, :], in_=ps, func=AF.Silu)

            # ---- xi in-projection (halo-rolled) ----
            for mt in range(6):
                if c > 0:
                    nc.gpsimd.tensor_copy(out=sb_xi[:, mt, 0:3],
                                          in_=sb_xi[:, mt, T:T + 3])
                ps = pA.tile([128, T], F32, tag="mm")
                for fh in range(2):
                    for kt in range(3):
                        nc.tensor.matmul(
                            ps[:, fh * 512:(fh + 1) * 512],
                            sb_wxi[:, kt, mt * 128:(mt + 1) * 128],
                            sb_x[:, kt, fh * 512:(fh + 1) * 512],
                            start=(kt == 0), stop=(kt == 2))
                act(out=sb_xi[:, mt, 3:T + 3], in_=ps, func=AF.Copy)

            # ---- conv taps + bias + silu -> xc ----
            sb_xc = p_xc.tile([128, 6, T], BF16, tag="xc")
            for dt in range(6):
                sb_tap = p_xdbl.tile([128, D_CONV, 128], BF16, tag="tap", bufs=2)
                nc.sync.dma_start(
                    out=sb_tap, in_=w_tap[:, dt * D_CONV:(dt + 1) * D_CONV, :])
                ps = pA.tile([128, T], F32, tag="mm")
                for fh in range(2):
                    for tap in range(D_CONV):
                        nc.tensor.matmul(
                            ps[:, fh * 512:(fh + 1) * 512],
                            sb_tap[:, tap, :],
                            sb_xi[:, dt, tap + fh * 512:tap + fh * 512 + 512],
                            start=(tap == 0), stop=(tap == D_CONV - 1))
                act(out=sb_xc[:, dt, :], in_=ps, func=AF.Silu,
                    bias=sb_cb[:, dt:dt + 1], scale=1.0)

            # ---- xproj -> xdbl; stage B/C rows ----
            psx = pA.tile([128, T], F32, tag="mm")
            for fh in range(2):
                for kt in range(6):
                    nc.tensor.matmul(
                        psx[0:NM, fh * 512:(fh + 1) * 512],
                        sb_wxp[:, kt, :],
                        sb_xc[:, kt, fh * 512:(fh + 1) * 512],
                        start=(kt == 0), stop=(kt == 5))
            sb_dl = p_dl.tile([128, 6, T], BF16, tag="dl")
            sb_xdbl = sb_dl[0:NM, 5, :]
            act(out=sb_xdbl, in_=psx[0:NM, :], func=AF.Copy, chain=False)
            nc.sync.dma_start(out=bc_stage[:, sl], in_=sb_dl[DT_RANK:NM, 5, :])

            sb_bb = p_bc.tile([128, NH, T], BF16, tag="bb")
            sb_cc = p_bc.tile([128, NH, T], BF16, tag="cc")
            for n in range(NH):
                nc.sync.dma_start(
                    out=sb_bb[:, n, :], in_=bc_stage[n, sl].partition_broadcast(128))
                nc.sync.dma_start(
                    out=sb_cc[:, n, :], in_=bc_stage[NH + n, sl].partition_broadcast(128))

            # ---- dt-proj -> softplus (exp in-place ln) -> dl ----
            for mt in range(6):
                ps = pA.tile([128, T], F32, tag="mm")
                for fh in range(2):
                    nc.tensor.matmul(
                        ps[:, fh * 512:(fh + 1) * 512],
                        sb_wdt[:, mt * 128:(mt + 1) * 128],
                        sb_dl[0:DT_RANK, 5, fh * 512:(fh + 1) * 512],
                        start=True, stop=True)
                act(out=sb_dl[:, mt, :], in_=ps, func=AF.Exp,
                    bias=sb_dtb[:, mt:mt + 1], scale=1.0)
            for mt in range(6):
                act(out=sb_dl[:, mt, :], in_=sb_dl[:, mt, :], func=AF.Ln,
                    bias=1.0, scale=1.0)

            # ---- per d-tile: da, dx, u, scan, q, py, gt ----
            for dt in range(6):
                sb_da = sb_das[dt % 2]
                sb_u = sb_us[dt % 2]
                for n in range(NH):
                    act(out=sb_da[:, n, 2:TP], in_=sb_dl[:, dt, :], func=AF.Exp,
                        scale=sb_asc[:, dt, n:n + 1])
                nc.gpsimd.tensor_tensor(out=sb_dx, in0=sb_dl[:, dt, :],
                                        in1=sb_xc[:, dt, :], op=OP.mult)
                nc.gpsimd.tensor_copy(out=sb_u[:, :, 0:1], in_=sb_carry[:, dt, :])
                nc.gpsimd.tensor_tensor(
                    out=sb_u[:, :, 2:TP], in0=mid_bcast(sb_dx, NH),
                    in1=sb_bb, op=OP.mult)
                nc.vector.tensor_tensor_scan(
                    out=sb_h.rearrange("p a b -> p (a b)"),
                    data0=sb_da.rearrange("p a b -> p (a b)"),
                    data1=sb_u.rearrange("p a b -> p (a b)"),
                    initial=0.0, op0=OP.mult, op1=OP.add)
                nc.gpsimd.tensor_copy(out=sb_carry[:, dt, :], in_=sb_h[:, :, TP - 1:TP])
                # q = h * C, into da's dead storage
                nc.vector.tensor_tensor(out=sb_da[:, :, 2:TP], in0=sb_h[:, :, 2:TP],
                                        in1=sb_cc, op=OP.mult)
                py = pPY.tile([128, T], F32, tag="py")
                for fh in range(2):
                    for n in range(NH):
                        nc.tensor.matmul(
                            py[:, fh * 512:(fh + 1) * 512], sb_id,
                            sb_da[:, n, 2 + fh * 512:2 + (fh + 1) * 512],
                            start=(n == 0), stop=False)
                    nc.tensor.matmul(
                        py[:, fh * 512:(fh + 1) * 512], sb_wdsk[:, dt, :],
                        sb_xc[:, dt, fh * 512:(fh + 1) * 512],
                        start=False, stop=True)
                # gt = py * sz, overwrite xc slice
                nc.vector.tensor_tensor(out=sb_xc[:, dt, :], in0=py,
                                        in1=sb_sz[:, dt, :], op=OP.mult)

            # ---- out_proj (ydm into sb_h's dead storage rows 0:3) ----
            for mt in range(3):
                ps = pA.tile([128, T], F32, tag="mm")
                for fh in range(2):
                    for kt in range(6):
                        nc.tensor.matmul(
                            ps[:, fh * 512:(fh + 1) * 512],
                            sb_wout[:, kt, mt * 128:(mt + 1) * 128],
                            sb_xc[:, kt, fh * 512:(fh + 1) * 512],
                            start=(kt == 0), stop=(kt == 5))
                act(out=sb_h[:, mt, 2:TP], in_=ps, func=AF.Copy, chain=False)

            # ---- z-conv partial (zc staged in bb's dead rows 0:6) ----
            for mt in range(6):
                ps = pA.tile([128, T], F32, tag="mm")
                for fh in range(2):
                    for kt in range(3):
                        nc.tensor.matmul(
                            ps[:, fh * 512:(fh + 1) * 512],
                            sb_wzc[:, kt, mt * 128:(mt + 1) * 128],
                            sb_h[:, kt, 2 + fh * 512:2 + (fh + 1) * 512],
                            start=(kt == 0), stop=(kt == 2))
                nc.vector.tensor_copy(out=sb_bb[:, mt, :], in_=ps)
            nc.sync.dma_start(
                out=z_parts[c][:].rearrange("(a p) b -> p a b", p=128),
                in_=sb_bb[:, 0:6, :])

            nc.gpsimd.collective_compute(
                "ReduceScatter", OP.add, replica_groups=RG,
                ins=[z_parts[c][:]], outs=[z_reds[c][:]])

    # ---------------- GLU + GN finalize ----------------
    with ExitStack() as ctx:
        fin = ctx.enter_context(tc.tile_pool(name="fin", bufs=1))
        psf = ctx.enter_context(tc.tile_pool(name="psf", bufs=1, space="PSUM"))

        sb_cba = fin.tile([96, 1], F32)
        nc.sync.dma_start(out=sb_cba, in_=cb_a[:])
        sb_cbb = fin.tile([96, 1], F32)
        nc.sync.dma_start(out=sb_cbb, in_=cb_b[:])
        sb_st = fin.tile([96, 2 * NCH], F32)
        glu = fin.tile([96, L], F32)
        for c in range(NCH):
            sl = slice(c * T, (c + 1) * T)
            fa = fin.tile([96, T], BF16, tag="fa", bufs=2)
            nc.sync.dma_start(out=fa, in_=z_reds[c][0:96, :])
            fb = fin.tile([96, T], BF16, tag="fb", bufs=2)
            nc.sync.dma_start(out=fb, in_=z_reds[c][96:192, :])
            nc.scalar.activation(out=fb, in_=fb, func=AF.Sigmoid,
                                 bias=sb_cbb[:, 0:1], scale=1.0)
            nc.vector.scalar_tensor_tensor(
                out=glu[:, sl], in0=fa, scalar=sb_cba[:, 0:1],
                in1=fb, op0=OP.add, op1=OP.mult)
            fs = fin.tile([96, T], BF16, tag="fs", bufs=2)
            nc.scalar.activation(out=fs, in_=glu[:, sl], func=AF.Copy,
                                 accum_out=sb_st[:, 2 * c:2 * c + 1])
            nc.scalar.activation(out=fs, in_=glu[:, sl], func=AF.Square,
                                 accum_out=sb_st[:, 2 * c + 1:2 * c + 2])

        st2 = fin.tile([96, 2], F32)
        nc.vector.tensor_tensor(out=st2, in0=sb_st[:, 0:2], in1=sb_st[:, 2:4],
                                op=OP.add)
        st3 = fin.tile([96, 2], F32)
        nc.vector.tensor_tensor(out=st3, in0=sb_st[:, 4:6], in1=sb_st[:, 6:8],
                                op=OP.add)
        stats = fin.tile([96, 2], F32)
        nc.vector.tensor_tensor(out=stats, in0=st2, in1=st3, op=OP.add)
        ones = fin.tile([96, 1], F32)
        nc.vector.memset(ones, 1.0)
        pss = psf.tile([1, 2], F32, tag="pss")
        nc.tensor.matmul(pss, ones, stats, start=True, stop=True)
        s_loc = fin.tile([1, 2], F32)
        nc.vector.tensor_copy(out=s_loc, in_=pss)
        nc.sync.dma_start(out=gn_in[:], in_=s_loc)
        nc.gpsimd.collective_compute(
            "AllReduce", OP.add, replica_groups=RG,
            ins=[gn_in[:]], outs=[gn_out[:]])
        s_glob = fin.tile([1, 2], F32)
        nc.sync.dma_start(out=s_glob, in_=gn_out[:])

        mu = fin.tile([1, 1], F32)
        nc.scalar.mul(out=mu, in_=s_glob[:, 0:1], mul=1.0 / GN_N)
        ms = fin.tile([1, 1], F32)
        nc.scalar.mul(out=ms, in_=s_glob[:, 1:2], mul=1.0 / GN_N)
        mu2 = fin.tile([1, 1], F32)
        nc.scalar.activation(out=mu2, in_=mu, func=AF.Square)
        var = fin.tile([1, 1], F32)
        nc.vector.tensor_tensor(out=var, in0=ms, in1=mu2, op=OP.subtract)
        eps_sb = fin.tile([1, 1], F32)
        nc.vector.memset(eps_sb, 1e-5)
        std = fin.tile([1, 1], F32)
        nc.scalar.activation(out=std, in_=var, func=AF.Sqrt,
                             bias=eps_sb[:, 0:1], scale=1.0)
        rstd = fin.tile([1, 1], F32)
        nc.vector.reciprocal(out=rstd, in_=std)
        mr = fin.tile([1, 2], F32)
        nc.gpsimd.tensor_copy(out=mr[:, 0:1], in_=mu)
        nc.gpsimd.tensor_copy(out=mr[:, 1:2], in_=rstd)
        nc.sync.dma_start(out=mr_dram[:], in_=mr)
        mr96 = fin.tile([96, 2], F32)
        nc.sync.dma_start(out=mr96, in_=mr_dram[0, :].partition_broadcast(96))

        sb_gnw = fin.tile([96, 1], F32)
        nc.sync.dma_start(out=sb_gnw, in_=gnw[:])
        sb_gnb = fin.tile([96, 1], F32)
        nc.sync.dma_start(out=sb_gnb, in_=gnb[:])

        scale = fin.tile([96, 1], F32)
        nc.vector.tensor_tensor(out=scale, in0=sb_gnw, in1=mr96[:, 1:2],
                                op=OP.mult)
        y1 = fin.tile([96, L], F32)
        nc.vector.tensor_scalar(out=y1, in0=glu, scalar1=mr96[:, 0:1],
                                scalar2=scale, op0=OP.subtract, op1=OP.mult)
        y2 = fin.tile([96, L], F32)
        nc.vector.tensor_scalar_add(out=y2, in0=y1, scalar1=sb_gnb[:, 0:1])
        nc.sync.dma_start(out=y_out[:], in_=y2)


# ======================= host side =======================

def _tiles_pmajor(w, p=128):
    r, cdim = w.shape
    return np.ascontiguousarray(
        w.reshape(r // p, p, cdim).transpose(1, 0, 2))


def _vec6(v):
    return np.ascontiguousarray(v.reshape(6, 128).T)


_PROG = None


def _get_prog():
    global _PROG
    if _PROG is None:
        _PROG = build_program()
    return _PROG


def make_in_maps(inputs):
    x = np.asarray(inputs['x'], np.float32)
    c_w = np.asarray(inputs['c_w'], np.float32)[:, :, 0]
    c_b = np.asarray(inputs['c_b'], np.float32)
    gn_w = np.asarray(inputs['gn_w'], np.float32)
    gn_b = np.asarray(inputs['gn_b'], np.float32)

    perm = []
    for r in range(4):
        perm += list(range(r * 96, (r + 1) * 96))
        perm += list(range(D_MODEL + r * 96, D_MODEL + (r + 1) * 96))
    perm = np.array(perm)
    c_w_p = c_w[perm]
    c_b_p = c_b[perm]

    in_maps = []
    for core in range(8):
        b, rem = divmod(core, 4)
        dirn, nh = divmod(rem, 2)
        rank = rem
        pref = 'f_' if dirn == 0 else 'b_'
        g = lambda k: np.asarray(inputs[pref + k], np.float32)

        x_bc = x[b] if dirn == 0 else np.ascontiguousarray(x[b, :, ::-1])
        in_w = g('in_w')                    # [1536, 384]
        cw = g('conv_w')[:, 0, :]           # [768, 4]
        wtap = np.zeros((128, 6 * D_CONV, 128), np.float32)
        for dt in range(6):
            for tap in range(D_CONV):
                np.fill_diagonal(wtap[:, dt * D_CONV + tap, :],
                                 cw[dt * 128:(dt + 1) * 128, tap])
        xproj_w = g('xproj_w')              # [56, 768]
        rows = np.concatenate([
            xproj_w[:DT_RANK],
            xproj_w[DT_RANK + nh * NH: DT_RANK + (nh + 1) * NH],
            xproj_w[DT_RANK + D_STATE + nh * NH: DT_RANK + D_STATE + (nh + 1) * NH],
        ], 0)                               # [40, 768]
        A = -np.exp(g('A_log'))             # [768, 16]
        Dp = g('D') if nh == 0 else np.zeros(D_INNER, np.float32)
        wdsk = np.zeros((128, 6, 128), np.float32)
        for dt in range(6):
            np.fill_diagonal(wdsk[:, dt, :], Dp[dt * 128:(dt + 1) * 128])
        wc_slice = c_w_p[:, dirn * D_MODEL:(dirn + 1) * D_MODEL]  # [768, 384]

        m = {
            'x_bc': _tiles_pmajor(np.ascontiguousarray(x_bc)).astype(bf),
            'w_zg': _tiles_pmajor(
                np.ascontiguousarray(in_w[D_INNER:].T)).astype(bf),
            'w_xi': _tiles_pmajor(
                np.ascontiguousarray(in_w[:D_INNER].T)).astype(bf),
            'w_tap': wtap.astype(bf),
            'w_xp': _tiles_pmajor(rows.T).astype(bf),
            'w_dt': np.ascontiguousarray(g('dt_w').T).astype(bf),
            'w_out': _tiles_pmajor(g('out_w').T).astype(bf),
            'w_zc': _tiles_pmajor(np.ascontiguousarray(wc_slice.T)).astype(bf),
            'w_dsk': wdsk.astype(bf),
            'conv_b': _vec6(g('conv_b')),
            'dt_b': _vec6(g('dt_b')),
            'a_sc': np.ascontiguousarray(
                A[:, nh * NH:(nh + 1) * NH].reshape(6, 128, NH).transpose(1, 0, 2)),
            'cb_a': np.ascontiguousarray(
                c_b_p[rank * 192: rank * 192 + 96].reshape(96, 1)),
            'cb_b': np.ascontiguousarray(
                c_b_p[rank * 192 + 96:(rank + 1) * 192].reshape(96, 1)),
            'gnw': np.ascontiguousarray(
                gn_w[rank * 96:(rank + 1) * 96].reshape(96, 1)),
            'gnb': np.ascontiguousarray(
                gn_b[rank * 96:(rank + 1) * 96].reshape(96, 1)),
        }
        in_maps.append(m)
    return in_maps


def kernel(**inputs):
    nc = _get_prog()
    in_maps = make_in_maps(inputs)
    res = run_bass_kernel_spmd(nc, in_maps, list(range(8)))
    outs = res.results
    out = np.zeros((B, D_MODEL, L), np.float32)
    for core in range(8):
        b, rank = divmod(core, 4)
        out[b, rank * 96:(rank + 1) * 96, :] = outs[core]['y_out']
    return out


if __name__ == "__main__":
    import reference as ref
    inputs = {k: np.asarray(v) for k, v in ref.setup_inputs().items()}
    got = kernel(**inputs)
    exp = np.asarray(ref.reference(**inputs))
    rel = np.linalg.norm(got - exp) / np.linalg.norm(exp)
    print("rel fro err:", rel)


# revision 13
# speedup vs baseline: 1.1782x; 1.1782x over previous
"""BiMamba Trainium2 kernel — 8-core SPMD, v2.

Sharding: core = b*4 + dir*2 + nh  (b: batch, dir: fwd/rev, nh: state half).
Each core runs the full mamba pipeline for its (b, dir) on all 768 inner
channels but only its 8 of 16 SSM states, pushes its partial through its
direction's half of the final 1x1 conv, then per-chunk ReduceScatter(add)
over its batch group of 4 cores sums partials and hands each core 192
(permuted) channels = 96 GLU `a` channels + their 96 `b` partners.
GLU + GroupNorm finish locally (tiny AllReduce for the GN stats).

v2: T=1024 time chunks; depthwise conv = xi in-projection + 4 diagonal
tap matmuls accumulated in PSUM; silu/exp/ln on Scalar straight from
PSUM; scan carry injected via a leading scan column; u=dx*B on GpSimd;
q=h*C on Vector; D-skip folded into the state-reduce as a diagonal
matmul; per-chunk ReduceScatter overlapped with later chunks.
"""
import numpy as np
import ml_dtypes

import concourse.bass as bass
import concourse.bacc as bacc_mod
import concourse.mybir as mybir
import concourse.tile as tile
from concourse.tile_rust import add_dep_helper
from concourse.bass_utils import run_bass_kernel_spmd

F32 = mybir.dt.float32
BF16 = mybir.dt.bfloat16
AF = mybir.ActivationFunctionType
OP = mybir.AluOpType

D_MODEL = 384
D_INNER = 768
D_STATE = 16
D_CONV = 4
DT_RANK = 24
B = 2
L = 4096
T = 1024                # time chunk
TP = T + 2              # carry col + pass-through col (4B-aligned data)
NCH = L // T
NH = 8                  # states per core
NM = 40                 # xproj out rows: 24 dt + 8 B + 8 C
RG = [[0, 1, 2, 3], [4, 5, 6, 7]]   # batch groups
GN_N = float(D_MODEL * L)

bf = ml_dtypes.bfloat16


def build_program():
    nc = bacc_mod.Bacc(num_devices=8)

    x_in = nc.dram_tensor("x_bc", [128, 3, L], BF16, kind="ExternalInput")
    w_zg = nc.dram_tensor("w_zg", [128, 3, D_INNER], BF16, kind="ExternalInput")
    w_xi = nc.dram_tensor("w_xi", [128, 3, D_INNER], BF16, kind="ExternalInput")
    w_tap = nc.dram_tensor("w_tap", [128, 6 * D_CONV, 128], BF16, kind="ExternalInput")
    w_xp = nc.dram_tensor("w_xp", [128, 6, NM], BF16, kind="ExternalInput")
    w_dt = nc.dram_tensor("w_dt", [DT_RANK, D_INNER], BF16, kind="ExternalInput")
    w_out = nc.dram_tensor("w_out", [128, 6, D_MODEL], BF16, kind="ExternalInput")
    w_zc = nc.dram_tensor("w_zc", [128, 3, 2 * D_MODEL], BF16, kind="ExternalInput")
    w_dsk = nc.dram_tensor("w_dsk", [128, 6, 128], BF16, kind="ExternalInput")
    conv_b = nc.dram_tensor("conv_b", [128, 6], F32, kind="ExternalInput")
    dt_b = nc.dram_tensor("dt_b", [128, 6], F32, kind="ExternalInput")
    a_sc = nc.dram_tensor("a_sc", [128, 6, NH], F32, kind="ExternalInput")
    cb_a = nc.dram_tensor("cb_a", [96, 1], F32, kind="ExternalInput")
    cb_b = nc.dram_tensor("cb_b", [96, 1], F32, kind="ExternalInput")
    gnw = nc.dram_tensor("gnw", [96, 1], F32, kind="ExternalInput")
    gnb = nc.dram_tensor("gnb", [96, 1], F32, kind="ExternalInput")
    y_out = nc.dram_tensor("y_out", [96, L], F32, kind="ExternalOutput")

    bc_stage = nc.dram_tensor("bc_stage", [2 * NH, L], BF16)
    z_parts = [nc.dram_tensor(f"z_part{c}", [D_INNER, T], BF16) for c in range(NCH)]
    z_reds = [nc.dram_tensor(f"z_red{c}", [192, T], BF16) for c in range(NCH)]
    glu_dram = nc.dram_tensor("glu_dram", [96, L], F32)
    gn_in = nc.dram_tensor("gn_in", [1, 2], F32)
    gn_out = nc.dram_tensor("gn_out", [1, 2], F32)
    mr_dram = nc.dram_tensor("mr_dram", [1, 2], F32)

    ident_dram = nc.inline_tensor(np.eye(128, dtype=bf), name="ident")

    with tile.TileContext(nc) as tc:
        _body(tc, nc, x_in, w_zg, w_xi, w_tap, w_xp, w_dt, w_out, w_zc, w_dsk,
              conv_b, dt_b, a_sc, cb_a, cb_b, gnw, gnb, y_out,
              bc_stage, z_parts, z_reds, glu_dram, gn_in, gn_out, mr_dram,
              ident_dram)
    if not nc.is_finalized():
        nc.finalize()
    return nc


def _body(tc, nc, x_in, w_zg, w_xi, w_tap, w_xp, w_dt, w_out, w_zc, w_dsk,
          conv_b, dt_b, a_sc, cb_a, cb_b, gnw, gnb, y_out,
          bc_stage, z_parts, z_reds, glu_dram, gn_in, gn_out, mr_dram,
          ident_dram):
    from contextlib import ExitStack

    with ExitStack() as ctx:
        singles = ctx.enter_context(tc.tile_pool(name="singles", bufs=1))
        sb_wzg = singles.tile([128, 3, D_INNER], BF16)
        nc.sync.dma_start(out=sb_wzg, in_=w_zg[:])
        sb_wxi = singles.tile([128, 3, D_INNER], BF16)
        nc.sync.dma_start(out=sb_wxi, in_=w_xi[:])
        sb_wxp = singles.tile([128, 6, NM], BF16)
        nc.sync.dma_start(out=sb_wxp, in_=w_xp[:])
        sb_wdt = singles.tile([DT_RANK, D_INNER], BF16)
        nc.sync.dma_start(out=sb_wdt, in_=w_dt[:])
        sb_wout = singles.tile([128, 6, D_MODEL], BF16)
        nc.sync.dma_start(out=sb_wout, in_=w_out[:])
        sb_wzc = singles.tile([128, 3, 2 * D_MODEL], BF16)
        nc.sync.dma_start(out=sb_wzc, in_=w_zc[:])
        sb_wdsk = singles.tile([128, 6, 128], BF16)
        nc.sync.dma_start(out=sb_wdsk, in_=w_dsk[:])
        sb_cb = singles.tile([128, 6], F32)
        nc.sync.dma_start(out=sb_cb, in_=conv_b[:])
        sb_dtb = singles.tile([128, 6], F32)
        nc.sync.dma_start(out=sb_dtb, in_=dt_b[:])
        sb_asc = singles.tile([128, 6, NH], F32)
        nc.sync.dma_start(out=sb_asc, in_=a_sc[:])
        sb_id = singles.tile([128, 128], BF16)
        nc.sync.dma_start(out=sb_id, in_=ident_dram[:])
        sb_carry = singles.tile([128, 6, NH], F32)
        nc.vector.memset(sb_carry, 0.0)

        pA = ctx.enter_context(tc.tile_pool(name="pA", bufs=2, space="PSUM"))
        pPY = ctx.enter_context(tc.tile_pool(name="pPY", bufs=2, space="PSUM"))

        p_x = ctx.enter_context(tc.tile_pool(name="p_x", bufs=1))
        p_sz = ctx.enter_context(tc.tile_pool(name="p_sz", bufs=2))
        p_xi = ctx.enter_context(tc.tile_pool(name="p_xi", bufs=1))
        p_xc = ctx.enter_context(tc.tile_pool(name="p_xc", bufs=1))
        p_xdbl = ctx.enter_context(tc.tile_pool(name="p_xdbl", bufs=1))
        p_dl = ctx.enter_context(tc.tile_pool(name="p_dl", bufs=1))
        p_bc = ctx.enter_context(tc.tile_pool(name="p_bc", bufs=1))
        p_scan = ctx.enter_context(tc.tile_pool(name="p_scan", bufs=1))

        def mid_bcast(ap2d, reps):
            return bass.AP(tensor=ap2d.tensor, offset=ap2d.offset,
                           ap=[ap2d.ap[0], [0, reps], ap2d.ap[1]])

        # xi rolling tile (halo in cols 0:3, persistent across chunks)
        sb_xi = p_xi.tile([128, 6, T + 3], BF16)
        nc.gpsimd.memset(sb_xi[:, :, 0:3], 0.0)

        # double-buffered scan tiles; col 0 of da stays 0 forever
        sb_das = [p_scan.tile([128, NH, TP], BF16, name=f"sb_da{i}")
                  for i in range(2)]
        sb_us = [p_scan.tile([128, NH, TP], BF16, name=f"sb_u{i}")
                 for i in range(2)]
        sb_h = p_scan.tile([128, NH, TP], BF16)
        sb_dx = p_scan.tile([128, T], BF16)
        for t_ in sb_das:
            nc.gpsimd.memset(t_[:, :, 0:1], 0.0)
            nc.gpsimd.memset(t_[:, :, 1:2], 1.0)
        for t_ in sb_us:
            nc.gpsimd.memset(t_[:, :, 1:2], 0.0)

        last_act = [None]

        def act(out, in_, func, chain=True, **kw):
            i = nc.scalar.activation(out=out, in_=in_, func=func, **kw)
            if chain and last_act[0] is not None:
                add_dep_helper(i.ins, last_act[0].ins, sync=False,
                               reason="ACT table grouping")
            if chain:
                last_act[0] = i
            return i

        for c in range(NCH):
            sl = slice(c * T, (c + 1) * T)
            last_act[0] = None   # allow cross-chunk S overlap

            sb_x = p_x.tile([128, 3, T], BF16, tag="x")
            nc.sync.dma_start(out=sb_x, in_=x_in[:, :, sl])

            # ---- z-gate -> silu -> sz ----
            sb_sz = p_sz.tile([128, 6, T], BF16, tag="sz")
            for mt in range(6):
                ps = pA.tile([128, T], F32, tag="mm")
                for fh in range(2):
                    for kt in range(3):
                        nc.tensor.matmul(
                            ps[:, fh * 512:(fh + 1) * 512],
                            sb_wzg[:, kt, mt * 128:(mt + 1) * 128],
                            sb_x[:, kt, fh * 512:(fh + 1) * 512],
                            start=(kt == 0), stop=(kt == 2))
                act(out=sb_sz[:, mt, :], in_=ps, func=AF.Silu)

            # ---- xi in-projection (halo-rolled) ----
            for mt in range(6):
                if c > 0:
                    nc.gpsimd.tensor_copy(out=sb_xi[:, mt, 0:3],
                                          in_=sb_xi[:, mt, T:T + 3])
                ps = pA.tile([128, T], F32, tag="mm")
                for fh in range(2):
                    for kt in range(3):
                        nc.tensor.matmul(
                            ps[:, fh * 512:(fh + 1) * 512],
                            sb_wxi[:, kt, mt * 128:(mt + 1) * 128],
                            sb_x[:, kt, fh * 512:(fh + 1) * 512],
                            start=(kt == 0), stop=(kt == 2))
                act(out=sb_xi[:, mt, 3:T + 3], in_=ps, func=AF.Copy)

            # ---- conv taps + bias + silu -> xc ----
            sb_xc = p_xc.tile([128, 6, T], BF16, tag="xc")
            for dt in range(6):
                sb_tap = p_xdbl.tile([128, D_CONV, 128], BF16, tag="tap", bufs=2)
                nc.sync.dma_start(
                    out=sb_tap, in_=w_tap[:, dt * D_CONV:(dt + 1) * D_CONV, :])
                ps = pA.tile([128, T], F32, tag="mm")
                for fh in range(2):
                    for tap in range(D_CONV):
                        nc.tensor.matmul(
                            ps[:, fh * 512:(fh + 1) * 512],
                            sb_tap[:, tap, :],
                            sb_xi[:, dt, tap + fh * 512:tap + fh * 512 + 512],
                            start=(tap == 0), stop=(tap == D_CONV - 1))
                act(out=sb_xc[:, dt, :], in_=ps, func=AF.Silu,
                    bias=sb_cb[:, dt:dt + 1], scale=1.0)

            # ---- xproj -> xdbl; stage B/C rows ----
            psx = pA.tile([128, T], F32, tag="mm")
            for fh in range(2):
                for kt in range(6):
                    nc.tensor.matmul(
                        psx[0:NM, fh * 512:(fh + 1) * 512],
                        sb_wxp[:, kt, :],
                        sb_xc[:, kt, fh * 512:(fh + 1) * 512],
                        start=(kt == 0), stop=(kt == 5))
            sb_dl = p_dl.tile([128, 6, T], BF16, tag="dl")
            sb_xdbl = sb_dl[0:NM, 5, :]
            act(out=sb_xdbl, in_=psx[0:NM, :], func=AF.Copy, chain=False)
            nc.sync.dma_start(out=bc_stage[:, sl], in_=sb_dl[DT_RANK:NM, 5, :])

            sb_bb = p_bc.tile([128, NH, T], BF16, tag="bb")
            sb_cc = p_bc.tile([128, NH, T], BF16, tag="cc")
            for n in range(NH):
                nc.sync.dma_start(
                    out=sb_bb[:, n, :], in_=bc_stage[n, sl].partition_broadcast(128))
                nc.sync.dma_start(
                    out=sb_cc[:, n, :], in_=bc_stage[NH + n, sl].partition_broadcast(128))

            # ---- dt-proj -> softplus (exp in-place ln) -> dl ----
            for mt in range(6):
                ps = pA.tile([128, T], F32, tag="mm")
                for fh in range(2):
                    nc.tensor.matmul(
                        ps[:, fh * 512:(fh + 1) * 512],
                        sb_wdt[:, mt * 128:(mt + 1) * 128],
                        sb_dl[0:DT_RANK, 5, fh * 512:(fh + 1) * 512],
                        start=True, stop=True)
                act(out=sb_dl[:, mt, :], in_=ps, func=AF.Exp,
                    bias=sb_dtb[:, mt:mt + 1], scale=1.0)
            for mt in range(6):
                act(out=sb_dl[:, mt, :], in_=sb_dl[:, mt, :], func=AF.Ln,
                    bias=1.0, scale=1.0)

            # ---- per d-tile: da, dx, u, scan, q, py, gt ----
            for dt in range(6):
                sb_da = sb_das[dt % 2]
                sb_u = sb_us[dt % 2]
                for n in range(NH):
                    act(out=sb_da[:, n, 2:TP], in_=sb_dl[:, dt, :], func=AF.Exp,
                        scale=sb_asc[:, dt, n:n + 1])
                nc.gpsimd.tensor_tensor(out=sb_dx, in0=sb_dl[:, dt, :],
                                        in1=sb_xc[:, dt, :], op=OP.mult)
                nc.gpsimd.tensor_copy(out=sb_u[:, :, 0:1], in_=sb_carry[:, dt, :])
                nc.vector.tensor_tensor(
                    out=sb_u[:, :, 2:TP], in0=mid_bcast(sb_dx, NH),
                    in1=sb_bb, op=OP.mult)
                nc.vector.tensor_tensor_scan(
                    out=sb_h.rearrange("p a b -> p (a b)"),
                    data0=sb_da.rearrange("p a b -> p (a b)"),
                    data1=sb_u.rearrange("p a b -> p (a b)"),
                    initial=0.0, op0=OP.mult, op1=OP.add)
                nc.gpsimd.tensor_copy(out=sb_carry[:, dt, :], in_=sb_h[:, :, TP - 1:TP])
                # q = h * C, into da's dead storage
                nc.vector.tensor_tensor(out=sb_da[:, :, 2:TP], in0=sb_h[:, :, 2:TP],
                                        in1=sb_cc, op=OP.mult)
                py = pPY.tile([128, T], F32, tag="py")
                for fh in range(2):
                    for n in range(NH):
                        nc.tensor.matmul(
                            py[:, fh * 512:(fh + 1) * 512], sb_id,
                            sb_da[:, n, 2 + fh * 512:2 + (fh + 1) * 512],
                            start=(n == 0), stop=False)
                    nc.tensor.matmul(
                        py[:, fh * 512:(fh + 1) * 512], sb_wdsk[:, dt, :],
                        sb_xc[:, dt, fh * 512:(fh + 1) * 512],
                        start=False, stop=True)
                # gt = py * sz, overwrite xc slice
                nc.vector.tensor_tensor(out=sb_xc[:, dt, :], in0=py,
                                        in1=sb_sz[:, dt, :], op=OP.mult)

            # ---- out_proj (ydm into sb_h's dead storage rows 0:3) ----
            for mt in range(3):
                ps = pA.tile([128, T], F32, tag="mm")
                for fh in range(2):
                    for kt in range(6):
                        nc.tensor.matmul(
                            ps[:, fh * 512:(fh + 1) * 512],
                            sb_wout[:, kt, mt * 128:(mt + 1) * 128],
                            sb_xc[:, kt, fh * 512:(fh + 1) * 512],
                            start=(kt == 0), stop=(kt == 5))
                act(out=sb_h[:, mt, 2:TP], in_=ps, func=AF.Copy, chain=False)

            # ---- z-conv partial (zc staged in bb's dead rows 0:6) ----
            for mt in range(6):
                ps = pA.tile([128, T], F32, tag="mm")
                for fh in range(2):
                    for kt in range(3):
                        nc.tensor.matmul(
                            ps[:, fh * 512:(fh + 1) * 512],
                            sb_wzc[:, kt, mt * 128:(mt + 1) * 128],
                            sb_h[:, kt, 2 + fh * 512:2 + (fh + 1) * 512],
                            start=(kt == 0), stop=(kt == 2))
                nc.vector.tensor_copy(out=sb_bb[:, mt, :], in_=ps)
            nc.sync.dma_start(
                out=z_parts[c][:].rearrange("(a p) b -> p a b", p=128),
                in_=sb_bb[:, 0:6, :])

            nc.gpsimd.collective_compute(
                "ReduceScatter", OP.add, replica_groups=RG,
                ins=[z_parts[c][:]], outs=[z_reds[c][:]])

    # ---------------- GLU + GN finalize ----------------
    with ExitStack() as ctx:
        fin = ctx.enter_context(tc.tile_pool(name="fin", bufs=1))
        psf = ctx.enter_context(tc.tile_pool(name="psf", bufs=1, space="PSUM"))

        sb_cba = fin.tile([96, 1], F32)
        nc.sync.dma_start(out=sb_cba, in_=cb_a[:])
        sb_cbb = fin.tile([96, 1], F32)
        nc.sync.dma_start(out=sb_cbb, in_=cb_b[:])
        sb_st = fin.tile([96, 2 * NCH], F32)
        glu = fin.tile([96, L], F32)
        for c in range(NCH):
            sl = slice(c * T, (c + 1) * T)
            fa = fin.tile([96, T], BF16, tag="fa", bufs=2)
            nc.sync.dma_start(out=fa, in_=z_reds[c][0:96, :])
            fb = fin.tile([96, T], BF16, tag="fb", bufs=2)
            nc.sync.dma_start(out=fb, in_=z_reds[c][96:192, :])
            nc.scalar.activation(out=fb, in_=fb, func=AF.Sigmoid,
                                 bias=sb_cbb[:, 0:1], scale=1.0)
            nc.vector.scalar_tensor_tensor(
                out=glu[:, sl], in0=fa, scalar=sb_cba[:, 0:1],
                in1=fb, op0=OP.add, op1=OP.mult)
            fs = fin.tile([96, T], BF16, tag="fs", bufs=2)
            nc.scalar.activation(out=fs, in_=glu[:, sl], func=AF.Copy,
                                 accum_out=sb_st[:, 2 * c:2 * c + 1])
            nc.scalar.activation(out=fs, in_=glu[:, sl], func=AF.Square,
                                 accum_out=sb_st[:, 2 * c + 1:2 * c + 2])

        st2 = fin.tile([96, 2], F32)
        nc.vector.tensor_tensor(out=st2, in0=sb_st[:, 0:2], in1=sb_st[:, 2:4],
                                op=OP.add)
        st3 = fin.tile([96, 2], F32)
        nc.vector.tensor_tensor(out=st3, in0=sb_st[:, 4:6], in1=sb_st[:, 6:8],
                                op=OP.add)
        stats = fin.tile([96, 2], F32)
        nc.vector.tensor_tensor(out=stats, in0=st2, in1=st3, op=OP.add)
        ones = fin.tile([96, 1], F32)
        nc.vector.memset(ones, 1.0)
        pss = psf.tile([1, 2], F32, tag="pss")
        nc.tensor.matmul(pss, ones, stats, start=True, stop=True)
        s_loc = fin.tile([1, 2], F32)
        nc.vector.tensor_copy(out=s_loc, in_=pss)
        nc.sync.dma_start(out=gn_in[:], in_=s_loc)
        nc.gpsimd.collective_compute(
            "AllReduce", OP.add, replica_groups=RG,
            ins=[gn_in[:]], outs=[gn_out[:]])
        s_glob = fin.tile([1, 2], F32)
        nc.sync.dma_start(out=s_glob, in_=gn_out[:])

        mu = fin.tile([1, 1], F32)
        nc.scalar.mul(out=mu, in_=s_glob[:, 0:1], mul=1.0 / GN_N)
        ms = fin.tile([1, 1], F32)
        nc.scalar.mul(out=ms, in_=s_glob[:, 1:2], mul=1.0 / GN_N)
        mu2 = fin.tile([1, 1], F32)
        nc.scalar.activation(out=mu2, in_=mu, func=AF.Square)
        var = fin.tile([1, 1], F32)
        nc.vector.tensor_tensor(out=var, in0=ms, in1=mu2, op=OP.subtract)
        eps_sb = fin.tile([1, 1], F32)
        nc.vector.memset(eps_sb, 1e-5)
        std = fin.tile([1, 1], F32)
        nc.scalar.activation(out=std, in_=var, func=AF.Sqrt,
                             bias=eps_sb[:, 0:1], scale=1.0)
        rstd = fin.tile([1, 1], F32)
        nc.vector.reciprocal(out=rstd, in_=std)
        mr = fin.tile([1, 2], F32)
        nc.gpsimd.tensor_copy(out=mr[:, 0:1], in_=mu)
        nc.gpsimd.tensor_copy(out=mr[:, 1:2], in_=rstd)
        nc.sync.dma_start(out=mr_dram[:], in_=mr)
        mr96 = fin.tile([96, 2], F32)
        nc.sync.dma_start(out=mr96, in_=mr_dram[0, :].partition_broadcast(96))

        sb_gnw = fin.tile([96, 1], F32)
        nc.sync.dma_start(out=sb_gnw, in_=gnw[:])
        sb_gnb = fin.tile([96, 1], F32)
        nc.sync.dma_start(out=sb_gnb, in_=gnb[:])

        scale = fin.tile([96, 1], F32)
        nc.vector.tensor_tensor(out=scale, in0=sb_gnw, in1=mr96[:, 1:2],
                                op=OP.mult)
        y1 = fin.tile([96, L], F32)
        nc.vector.tensor_scalar(out=y1, in0=glu, scalar1=mr96[:, 0:1],
                                scalar2=scale, op0=OP.subtract, op1=OP.mult)
        y2 = fin.tile([96, L], F32)
        nc.vector.tensor_scalar_add(out=y2, in0=y1, scalar1=sb_gnb[:, 0:1])
        nc.sync.dma_start(out=y_out[:], in_=y2)


# ======================= host side =======================

def _tiles_pmajor(w, p=128):
    r, cdim = w.shape
    return np.ascontiguousarray(
        w.reshape(r // p, p, cdim).transpose(1, 0, 2))


def _vec6(v):
    return np.ascontiguousarray(v.reshape(6, 128).T)


_PROG = None


def _get_prog():
    global _PROG
    if _PROG is None:
        _PROG = build_program()
    return _PROG


def make_in_maps(inputs):
    x = np.asarray(inputs['x'], np.float32)
    c_w = np.asarray(inputs['c_w'], np.float32)[:, :, 0]
    c_b = np.asarray(inputs['c_b'], np.float32)
    gn_w = np.asarray(inputs['gn_w'], np.float32)
    gn_b = np.asarray(inputs['gn_b'], np.float32)

    perm = []
    for r in range(4):
        perm += list(range(r * 96, (r + 1) * 96))
        perm += list(range(D_MODEL + r * 96, D_MODEL + (r + 1) * 96))
    perm = np.array(perm)
    c_w_p = c_w[perm]
    c_b_p = c_b[perm]

    in_maps = []
    for core in range(8):
        b, rem = divmod(core, 4)
        dirn, nh = divmod(rem, 2)
        rank = rem
        pref = 'f_' if dirn == 0 else 'b_'
        g = lambda k: np.asarray(inputs[pref + k], np.float32)

        x_bc = x[b] if dirn == 0 else np.ascontiguousarray(x[b, :, ::-1])
        in_w = g('in_w')                    # [1536, 384]
        cw = g('conv_w')[:, 0, :]           # [768, 4]
        wtap = np.zeros((128, 6 * D_CONV, 128), np.float32)
        for dt in range(6):
            for tap in range(D_CONV):
                np.fill_diagonal(wtap[:, dt * D_CONV + tap, :],
                                 cw[dt * 128:(dt + 1) * 128, tap])
        xproj_w = g('xproj_w')              # [56, 768]
        rows = np.concatenate([
            xproj_w[:DT_RANK],
            xproj_w[DT_RANK + nh * NH: DT_RANK + (nh + 1) * NH],
            xproj_w[DT_RANK + D_STATE + nh * NH: DT_RANK + D_STATE + (nh + 1) * NH],
        ], 0)                               # [40, 768]
        A = -np.exp(g('A_log'))             # [768, 16]
        Dp = g('D') if nh == 0 else np.zeros(D_INNER, np.float32)
        wdsk = np.zeros((128, 6, 128), np.float32)
        for dt in range(6):
            np.fill_diagonal(wdsk[:, dt, :], Dp[dt * 128:(dt + 1) * 128])
        wc_slice = c_w_p[:, dirn * D_MODEL:(dirn + 1) * D_MODEL]  # [768, 384]

        m = {
            'x_bc': _tiles_pmajor(np.ascontiguousarray(x_bc)).astype(bf),
            'w_zg': _tiles_pmajor(
                np.ascontiguousarray(in_w[D_INNER:].T)).astype(bf),
            'w_xi': _tiles_pmajor(
                np.ascontiguousarray(in_w[:D_INNER].T)).astype(bf),
            'w_tap': wtap.astype(bf),
            'w_xp': _tiles_pmajor(rows.T).astype(bf),
            'w_dt': np.ascontiguousarray(g('dt_w').T).astype(bf),
            'w_out': _tiles_pmajor(g('out_w').T).astype(bf),
            'w_zc': _tiles_pmajor(np.ascontiguousarray(wc_slice.T)).astype(bf),
            'w_dsk': wdsk.astype(bf),
            'conv_b': _vec6(g('conv_b')),
            'dt_b': _vec6(g('dt_b')),
            'a_sc': np.ascontiguousarray(
                A[:, nh * NH:(nh + 1) * NH].reshape(6, 128, NH).transpose(1, 0, 2)),
            'cb_a': np.ascontiguousarray(
                c_b_p[rank * 192: rank * 192 + 96].reshape(96, 1)),
            'cb_b': np.ascontiguousarray(
                c_b_p[rank * 192 + 96:(rank + 1) * 192].reshape(96, 1)),
            'gnw': np.ascontiguousarray(
                gn_w[rank * 96:(rank + 1) * 96].reshape(96, 1)),
            'gnb': np.ascontiguousarray(
                gn_b[rank * 96:(rank + 1) * 96].reshape(96, 1)),
        }
        in_maps.append(m)
    return in_maps


def kernel(**inputs):
    nc = _get_prog()
    in_maps = make_in_maps(inputs)
    res = run_bass_kernel_spmd(nc, in_maps, list(range(8)))
    outs = res.results
    out = np.zeros((B, D_MODEL, L), np.float32)
    for core in range(8):
        b, rank = divmod(core, 4)
        out[b, rank * 96:(rank + 1) * 96, :] = outs[core]['y_out']
    return out


if __name__ == "__main__":
    import reference as ref
    inputs = {k: np.asarray(v) for k, v in ref.setup_inputs().items()}
    got = kernel(**inputs)
    exp = np.asarray(ref.reference(**inputs))
    rel = np.linalg.norm(got - exp) / np.linalg.norm(exp)
    print("rel fro err:", rel)


# revision 15
# speedup vs baseline: 1.1932x; 1.0127x over previous
"""BiMamba Trainium2 kernel — 8-core SPMD, v2.

Sharding: core = b*4 + dir*2 + nh  (b: batch, dir: fwd/rev, nh: state half).
Each core runs the full mamba pipeline for its (b, dir) on all 768 inner
channels but only its 8 of 16 SSM states, pushes its partial through its
direction's half of the final 1x1 conv, then per-chunk ReduceScatter(add)
over its batch group of 4 cores sums partials and hands each core 192
(permuted) channels = 96 GLU `a` channels + their 96 `b` partners.
GLU + GroupNorm finish locally (tiny AllReduce for the GN stats).

v2: T=1024 time chunks; depthwise conv = xi in-projection + 4 diagonal
tap matmuls accumulated in PSUM; silu/exp/ln on Scalar straight from
PSUM; scan carry injected via a leading scan column; u=dx*B on GpSimd;
q=h*C on Vector; D-skip folded into the state-reduce as a diagonal
matmul; per-chunk ReduceScatter overlapped with later chunks.
"""
import numpy as np
import ml_dtypes

import concourse.bass as bass
import concourse.bacc as bacc_mod
import concourse.mybir as mybir
import concourse.tile as tile
from concourse.tile_rust import add_dep_helper
from concourse.bass_utils import run_bass_kernel_spmd

F32 = mybir.dt.float32
BF16 = mybir.dt.bfloat16
AF = mybir.ActivationFunctionType
OP = mybir.AluOpType

D_MODEL = 384
D_INNER = 768
D_STATE = 16
D_CONV = 4
DT_RANK = 24
B = 2
L = 4096
T = 1024                # time chunk
TP = T + 2              # carry col + pass-through col (4B-aligned data)
NCH = L // T
NH = 8                  # states per core
NM = 40                 # xproj out rows: 24 dt + 8 B + 8 C
RG = [[0, 1, 2, 3], [4, 5, 6, 7]]   # batch groups
GN_N = float(D_MODEL * L)

bf = ml_dtypes.bfloat16


def build_program():
    nc = bacc_mod.Bacc(num_devices=8)

    x_in = nc.dram_tensor("x_bc", [128, 3, L], BF16, kind="ExternalInput")
    w_zg = nc.dram_tensor("w_zg", [128, 3, D_INNER], BF16, kind="ExternalInput")
    w_xi = nc.dram_tensor("w_xi", [128, 3, D_INNER], BF16, kind="ExternalInput")
    w_tap = nc.dram_tensor("w_tap", [128, 6 * D_CONV, 128], BF16, kind="ExternalInput")
    w_xp = nc.dram_tensor("w_xp", [128, 6, NM], BF16, kind="ExternalInput")
    w_dt = nc.dram_tensor("w_dt", [DT_RANK, D_INNER], BF16, kind="ExternalInput")
    w_out = nc.dram_tensor("w_out", [128, 6, D_MODEL], BF16, kind="ExternalInput")
    w_zc = nc.dram_tensor("w_zc", [128, 3, 2 * D_MODEL], BF16, kind="ExternalInput")
    w_dsk = nc.dram_tensor("w_dsk", [128, 6, 128], BF16, kind="ExternalInput")
    conv_b = nc.dram_tensor("conv_b", [128, 6], F32, kind="ExternalInput")
    dt_b = nc.dram_tensor("dt_b", [128, 6], F32, kind="ExternalInput")
    a_sc = nc.dram_tensor("a_sc", [128, 6, NH], F32, kind="ExternalInput")
    cb_a = nc.dram_tensor("cb_a", [96, 1], F32, kind="ExternalInput")
    cb_b = nc.dram_tensor("cb_b", [96, 1], F32, kind="ExternalInput")
    gnw = nc.dram_tensor("gnw", [96, 1], F32, kind="ExternalInput")
    gnb = nc.dram_tensor("gnb", [96, 1], F32, kind="ExternalInput")
    y_out = nc.dram_tensor("y_out", [96, L], F32, kind="ExternalOutput")

    bc_stage = nc.dram_tensor("bc_stage", [2 * NH, L], BF16)
    z_parts = [nc.dram_tensor(f"z_part{c}", [D_INNER, T], BF16) for c in range(NCH)]
    z_reds = [nc.dram_tensor(f"z_red{c}", [192, T], BF16) for c in range(NCH)]
    glu_dram = nc.dram_tensor("glu_dram", [96, L], BF16)
    gn_in = nc.dram_tensor("gn_in", [1, 2], F32)
    gn_out = nc.dram_tensor("gn_out", [1, 2], F32)
    mr_dram = nc.dram_tensor("mr_dram", [1, 2], F32)

    ident_dram = nc.inline_tensor(np.eye(128, dtype=bf), name="ident")

    with tile.TileContext(nc) as tc:
        _body(tc, nc, x_in, w_zg, w_xi, w_tap, w_xp, w_dt, w_out, w_zc, w_dsk,
              conv_b, dt_b, a_sc, cb_a, cb_b, gnw, gnb, y_out,
              bc_stage, z_parts, z_reds, glu_dram, gn_in, gn_out, mr_dram,
              ident_dram)
    if not nc.is_finalized():
        nc.finalize()
    return nc


def _body(tc, nc, x_in, w_zg, w_xi, w_tap, w_xp, w_dt, w_out, w_zc, w_dsk,
          conv_b, dt_b, a_sc, cb_a, cb_b, gnw, gnb, y_out,
          bc_stage, z_parts, z_reds, glu_dram, gn_in, gn_out, mr_dram,
          ident_dram):
    from contextlib import ExitStack

    with ExitStack() as ctx:
        singles = ctx.enter_context(tc.tile_pool(name="singles", bufs=1))
        sb_wzg = singles.tile([128, 3, D_INNER], BF16)
        nc.sync.dma_start(out=sb_wzg, in_=w_zg[:])
        sb_wxi = singles.tile([128, 3, D_INNER], BF16)
        nc.sync.dma_start(out=sb_wxi, in_=w_xi[:])
        sb_wxp = singles.tile([128, 6, NM], BF16)
        nc.sync.dma_start(out=sb_wxp, in_=w_xp[:])
        sb_wdt = singles.tile([DT_RANK, D_INNER], BF16)
        nc.sync.dma_start(out=sb_wdt, in_=w_dt[:])
        sb_wout = singles.tile([128, 6, D_MODEL], BF16)
        nc.sync.dma_start(out=sb_wout, in_=w_out[:])
        sb_wzc = singles.tile([128, 3, 2 * D_MODEL], BF16)
        nc.sync.dma_start(out=sb_wzc, in_=w_zc[:])
        sb_wdsk = singles.tile([128, 6, 128], BF16)
        nc.sync.dma_start(out=sb_wdsk, in_=w_dsk[:])
        sb_cb = singles.tile([128, 6], F32)
        nc.sync.dma_start(out=sb_cb, in_=conv_b[:])
        sb_dtb = singles.tile([128, 6], F32)
        nc.sync.dma_start(out=sb_dtb, in_=dt_b[:])
        sb_asc = singles.tile([128, 6, NH], F32)
        nc.sync.dma_start(out=sb_asc, in_=a_sc[:])
        sb_id = singles.tile([128, 128], BF16)
        nc.sync.dma_start(out=sb_id, in_=ident_dram[:])
        sb_cba = singles.tile([96, 1], F32)
        nc.sync.dma_start(out=sb_cba, in_=cb_a[:])
        sb_cbb = singles.tile([96, 1], F32)
        nc.sync.dma_start(out=sb_cbb, in_=cb_b[:])
        sb_st = singles.tile([96, 2 * NCH], F32)
        sb_carry = singles.tile([128, 6, NH], F32)
        nc.vector.memset(sb_carry, 0.0)

        pA = ctx.enter_context(tc.tile_pool(name="pA", bufs=2, space="PSUM"))
        pPY = ctx.enter_context(tc.tile_pool(name="pPY", bufs=2, space="PSUM"))

        p_x = ctx.enter_context(tc.tile_pool(name="p_x", bufs=1))
        p_sz = ctx.enter_context(tc.tile_pool(name="p_sz", bufs=1))
        p_xi = ctx.enter_context(tc.tile_pool(name="p_xi", bufs=1))
        p_xc = ctx.enter_context(tc.tile_pool(name="p_xc", bufs=2))
        p_xdbl = ctx.enter_context(tc.tile_pool(name="p_xdbl", bufs=1))
        p_dl = ctx.enter_context(tc.tile_pool(name="p_dl", bufs=2))
        p_bc = ctx.enter_context(tc.tile_pool(name="p_bc", bufs=1))
        p_scan = ctx.enter_context(tc.tile_pool(name="p_scan", bufs=1))

        def mid_bcast(ap2d, reps):
            return bass.AP(tensor=ap2d.tensor, offset=ap2d.offset,
                           ap=[ap2d.ap[0], [0, reps], ap2d.ap[1]])

        # xi rolling tile (halo in cols 0:3, persistent across chunks)
        sb_xi = p_xi.tile([128, 6, T + 3], BF16)
        nc.gpsimd.memset(sb_xi[:, :, 0:3], 0.0)

        # double-buffered scan tiles; col 0 of da stays 0 forever
        sb_das = [p_scan.tile([128, NH, TP], BF16, name=f"sb_da{i}")
                  for i in range(2)]
        sb_u = p_scan.tile([128, NH, TP], BF16)
        sb_h = p_scan.tile([128, NH, TP], BF16)
        sb_dx = p_scan.tile([128, T], BF16)
        for t_ in sb_das:
            nc.gpsimd.memset(t_[:, :, 0:1], 0.0)
            nc.gpsimd.memset(t_[:, :, 1:2], 1.0)
        nc.gpsimd.memset(sb_u[:, :, 1:2], 0.0)

        last_act = [None]
        last_z = [None]

        def act(out, in_, func, chain=True, zchain=False, **kw):
            i = nc.scalar.activation(out=out, in_=in_, func=func, **kw)
            ref = last_z if zchain else last_act
            if (chain or zchain) and ref[0] is not None:
                add_dep_helper(i.ins, ref[0].ins, sync=False,
                               reason="ACT table grouping")
            if chain or zchain:
                ref[0] = i
            return i

        for c in range(NCH):
            sl = slice(c * T, (c + 1) * T)
            last_act[0] = None   # allow cross-chunk S overlap

            sb_x = p_x.tile([128, 3, T], BF16, tag="x")
            nc.sync.dma_start(out=sb_x, in_=x_in[:, :, sl])

            # ---- z-gate -> silu -> sz ----
            sb_sz = p_sz.tile([128, 6, T], BF16, tag="sz")
            for mt in range(6):
                ps = pA.tile([128, T], F32, tag="mm")
                for fh in range(2):
                    for kt in range(3):
                        nc.tensor.matmul(
                            ps[:, fh * 512:(fh + 1) * 512],
                            sb_wzg[:, kt, mt * 128:(mt + 1) * 128],
                            sb_x[:, kt, fh * 512:(fh + 1) * 512],
                            start=(kt == 0), stop=(kt == 2))
                act(out=sb_sz[:, mt, :], in_=ps, func=AF.Silu,
                    chain=False, zchain=True)

            # ---- xi in-projection (halo-rolled) ----
            for mt in range(6):
                if c > 0:
                    nc.gpsimd.tensor_copy(out=sb_xi[:, mt, 0:3],
                                          in_=sb_xi[:, mt, T:T + 3])
                ps = pA.tile([128, T], F32, tag="mm")
                for fh in range(2):
                    for kt in range(3):
                        nc.tensor.matmul(
                            ps[:, fh * 512:(fh + 1) * 512],
                            sb_wxi[:, kt, mt * 128:(mt + 1) * 128],
                            sb_x[:, kt, fh * 512:(fh + 1) * 512],
                            start=(kt == 0), stop=(kt == 2))
                act(out=sb_xi[:, mt, 3:T + 3], in_=ps, func=AF.Copy)

            # ---- conv taps + bias + silu -> xc ----
            sb_xc = p_xc.tile([128, 6, T], BF16, tag="xc")
            for dt in range(6):
                sb_tap = p_xdbl.tile([128, D_CONV, 128], BF16, tag="tap", bufs=2)
                nc.sync.dma_start(
                    out=sb_tap, in_=w_tap[:, dt * D_CONV:(dt + 1) * D_CONV, :])
                ps = pA.tile([128, T], F32, tag="mm")
                for fh in range(2):
                    for tap in range(D_CONV):
                        nc.tensor.matmul(
                            ps[:, fh * 512:(fh + 1) * 512],
                            sb_tap[:, tap, :],
                            sb_xi[:, dt, tap + fh * 512:tap + fh * 512 + 512],
                            start=(tap == 0), stop=(tap == D_CONV - 1))
                act(out=sb_xc[:, dt, :], in_=ps, func=AF.Silu,
                    bias=sb_cb[:, dt:dt + 1], scale=1.0)

            # ---- xproj -> xdbl; stage B/C rows ----
            psx = pA.tile([128, T], F32, tag="mm")
            for fh in range(2):
                for kt in range(6):
                    nc.tensor.matmul(
                        psx[0:NM, fh * 512:(fh + 1) * 512],
                        sb_wxp[:, kt, :],
                        sb_xc[:, kt, fh * 512:(fh + 1) * 512],
                        start=(kt == 0), stop=(kt == 5))
            sb_dl = p_dl.tile([128, 6, T], BF16, tag="dl")
            sb_xdbl = sb_dl[0:NM, 5, :]
            act(out=sb_xdbl, in_=psx[0:NM, :], func=AF.Copy, chain=False)
            nc.sync.dma_start(out=bc_stage[:, sl], in_=sb_dl[DT_RANK:NM, 5, :])

            sb_bb = p_bc.tile([128, NH, T], BF16, tag="bb")
            sb_cc = p_bc.tile([128, NH, T], BF16, tag="cc")
            for n in range(NH):
                nc.sync.dma_start(
                    out=sb_bb[:, n, :], in_=bc_stage[n, sl].partition_broadcast(128))
                nc.sync.dma_start(
                    out=sb_cc[:, n, :], in_=bc_stage[NH + n, sl].partition_broadcast(128))

            # ---- dt-proj -> softplus (exp in-place ln) -> dl ----
            for mt in range(6):
                ps = pA.tile([128, T], F32, tag="mm")
                for fh in range(2):
                    nc.tensor.matmul(
                        ps[:, fh * 512:(fh + 1) * 512],
                        sb_wdt[:, mt * 128:(mt + 1) * 128],
                        sb_dl[0:DT_RANK, 5, fh * 512:(fh + 1) * 512],
                        start=True, stop=True)
                act(out=sb_dl[:, mt, :], in_=ps, func=AF.Exp,
                    bias=sb_dtb[:, mt:mt + 1], scale=1.0)
            for mt in range(6):
                act(out=sb_dl[:, mt, :], in_=sb_dl[:, mt, :], func=AF.Ln,
                    bias=1.0, scale=1.0)

            # ---- per d-tile: da, dx, u, scan, q, py, gt ----
            for dt in range(6):
                sb_da = sb_das[dt % 2]
                for n in range(NH):
                    act(out=sb_da[:, n, 2:TP], in_=sb_dl[:, dt, :], func=AF.Exp,
                        scale=sb_asc[:, dt, n:n + 1])
                nc.gpsimd.tensor_tensor(out=sb_dx, in0=sb_dl[:, dt, :],
                                        in1=sb_xc[:, dt, :], op=OP.mult)
                nc.gpsimd.tensor_copy(out=sb_u[:, :, 0:1], in_=sb_carry[:, dt, :])
                nc.vector.tensor_tensor(
                    out=sb_u[:, :, 2:TP], in0=mid_bcast(sb_dx, NH),
                    in1=sb_bb, op=OP.mult)
                nc.vector.tensor_tensor_scan(
                    out=sb_h.rearrange("p a b -> p (a b)"),
                    data0=sb_da.rearrange("p a b -> p (a b)"),
                    data1=sb_u.rearrange("p a b -> p (a b)"),
                    initial=0.0, op0=OP.mult, op1=OP.add)
                nc.gpsimd.tensor_copy(out=sb_carry[:, dt, :], in_=sb_h[:, :, TP - 1:TP])
                # q = h * C, into da's dead storage
                nc.vector.tensor_tensor(out=sb_da[:, :, 2:TP], in0=sb_h[:, :, 2:TP],
                                        in1=sb_cc, op=OP.mult)
                py = pPY.tile([128, T], F32, tag="py")
                for fh in range(2):
                    for n in range(NH):
                        nc.tensor.matmul(
                            py[:, fh * 512:(fh + 1) * 512], sb_id,
                            sb_da[:, n, 2 + fh * 512:2 + (fh + 1) * 512],
                            start=(n == 0), stop=False)
                    nc.tensor.matmul(
                        py[:, fh * 512:(fh + 1) * 512], sb_wdsk[:, dt, :],
                        sb_xc[:, dt, fh * 512:(fh + 1) * 512],
                        start=False, stop=True)
                # gt = py * sz, overwrite xc slice
                nc.vector.tensor_tensor(out=sb_xc[:, dt, :], in0=py,
                                        in1=sb_sz[:, dt, :], op=OP.mult)

            # ---- out_proj (ydm into sb_h's dead storage rows 0:3) ----
            for mt in range(3):
                ps = pA.tile([128, T], F32, tag="mm")
                for fh in range(2):
                    for kt in range(6):
                        nc.tensor.matmul(
                            ps[:, fh * 512:(fh + 1) * 512],
                            sb_wout[:, kt, mt * 128:(mt + 1) * 128],
                            sb_xc[:, kt, fh * 512:(fh + 1) * 512],
                            start=(kt == 0), stop=(kt == 5))
                act(out=sb_h[:, mt, 2:TP], in_=ps, func=AF.Copy, chain=False)

            # ---- z-conv partial (zc staged in bb's dead rows 0:6) ----
            for mt in range(6):
                ps = pA.tile([128, T], F32, tag="mm")
                for fh in range(2):
                    for kt in range(3):
                        nc.tensor.matmul(
                            ps[:, fh * 512:(fh + 1) * 512],
                            sb_wzc[:, kt, mt * 128:(mt + 1) * 128],
                            sb_h[:, kt, 2 + fh * 512:2 + (fh + 1) * 512],
                            start=(kt == 0), stop=(kt == 2))
                nc.vector.tensor_copy(out=sb_bb[:, mt, :], in_=ps)
            nc.sync.dma_start(
                out=z_parts[c][:].rearrange("(a p) b -> p a b", p=128),
                in_=sb_bb[:, 0:6, :])

            nc.gpsimd.collective_compute(
                "ReduceScatter", OP.add, replica_groups=RG,
                ins=[z_parts[c][:]], outs=[z_reds[c][:]])

            fa = p_x.tile([96, T], BF16, tag="fa", bufs=1)
            nc.sync.dma_start(out=fa, in_=z_reds[c][0:96, :])
            fb = p_x.tile([96, T], BF16, tag="fb", bufs=1)
            nc.sync.dma_start(out=fb, in_=z_reds[c][96:192, :])
            act(out=fb, in_=fb, func=AF.Sigmoid,
                bias=sb_cbb[:, 0:1], scale=1.0, chain=False)
            go = p_x.tile([96, T], BF16, tag="go", bufs=1)
            nc.vector.scalar_tensor_tensor(
                out=go, in0=fa, scalar=sb_cba[:, 0:1],
                in1=fb, op0=OP.add, op1=OP.mult)
            nc.sync.dma_start(out=glu_dram[:, sl], in_=go)
            act(out=fa, in_=go, func=AF.Copy,
                accum_out=sb_st[:, 2 * c:2 * c + 1], chain=False)
            act(out=fa, in_=go, func=AF.Square,
                accum_out=sb_st[:, 2 * c + 1:2 * c + 2], chain=False)

    # ---------------- GLU + GN finalize ----------------
    with ExitStack() as ctx:
        fin = ctx.enter_context(tc.tile_pool(name="fin", bufs=1))
        psf = ctx.enter_context(tc.tile_pool(name="psf", bufs=1, space="PSUM"))

        glu = fin.tile([96, L], BF16)
        nc.sync.dma_start(out=glu, in_=glu_dram[:])
        st2 = fin.tile([96, 2], F32)
        nc.vector.tensor_tensor(out=st2, in0=sb_st[:, 0:2], in1=sb_st[:, 2:4],
                                op=OP.add)
        st3 = fin.tile([96, 2], F32)
        nc.vector.tensor_tensor(out=st3, in0=sb_st[:, 4:6], in1=sb_st[:, 6:8],
                                op=OP.add)
        stats = fin.tile([96, 2], F32)
        nc.vector.tensor_tensor(out=stats, in0=st2, in1=st3, op=OP.add)
        ones = fin.tile([96, 1], F32)
        nc.vector.memset(ones, 1.0)
        pss = psf.tile([1, 2], F32, tag="pss")
        nc.tensor.matmul(pss, ones, stats, start=True, stop=True)
        s_loc = fin.tile([1, 2], F32)
        nc.vector.tensor_copy(out=s_loc, in_=pss)
        nc.sync.dma_start(out=gn_in[:], in_=s_loc)
        nc.gpsimd.collective_compute(
            "AllReduce", OP.add, replica_groups=RG,
            ins=[gn_in[:]], outs=[gn_out[:]])
        s_glob = fin.tile([1, 2], F32)
        nc.sync.dma_start(out=s_glob, in_=gn_out[:])

        mu = fin.tile([1, 1], F32)
        nc.scalar.mul(out=mu, in_=s_glob[:, 0:1], mul=1.0 / GN_N)
        ms = fin.tile([1, 1], F32)
        nc.scalar.mul(out=ms, in_=s_glob[:, 1:2], mul=1.0 / GN_N)
        mu2 = fin.tile([1, 1], F32)
        nc.scalar.activation(out=mu2, in_=mu, func=AF.Square)
        var = fin.tile([1, 1], F32)
        nc.vector.tensor_tensor(out=var, in0=ms, in1=mu2, op=OP.subtract)
        eps_sb = fin.tile([1, 1], F32)
        nc.vector.memset(eps_sb, 1e-5)
        std = fin.tile([1, 1], F32)
        nc.scalar.activation(out=std, in_=var, func=AF.Sqrt,
                             bias=eps_sb[:, 0:1], scale=1.0)
        rstd = fin.tile([1, 1], F32)
        nc.vector.reciprocal(out=rstd, in_=std)
        mr = fin.tile([1, 2], F32)
        nc.gpsimd.tensor_copy(out=mr[:, 0:1], in_=mu)
        nc.gpsimd.tensor_copy(out=mr[:, 1:2], in_=rstd)
        nc.sync.dma_start(out=mr_dram[:], in_=mr)
        mr96 = fin.tile([96, 2], F32)
        nc.sync.dma_start(out=mr96, in_=mr_dram[0, :].partition_broadcast(96))

        sb_gnw = fin.tile([96, 1], F32)
        nc.sync.dma_start(out=sb_gnw, in_=gnw[:])
        sb_gnb = fin.tile([96, 1], F32)
        nc.sync.dma_start(out=sb_gnb, in_=gnb[:])

        scale = fin.tile([96, 1], F32)
        nc.vector.tensor_tensor(out=scale, in0=sb_gnw, in1=mr96[:, 1:2],
                                op=OP.mult)
        y1 = fin.tile([96, L], F32)
        nc.vector.tensor_scalar(out=y1, in0=glu, scalar1=mr96[:, 0:1],
                                scalar2=scale, op0=OP.subtract, op1=OP.mult)
        y2 = fin.tile([96, L], F32)
        nc.vector.tensor_scalar_add(out=y2, in0=y1, scalar1=sb_gnb[:, 0:1])
        nc.sync.dma_start(out=y_out[:], in_=y2)


# ======================= host side =======================

def _tiles_pmajor(w, p=128):
    r, cdim = w.shape
    return np.ascontiguousarray(
        w.reshape(r // p, p, cdim).transpose(1, 0, 2))


def _vec6(v):
    return np.ascontiguousarray(v.reshape(6, 128).T)


_PROG = None


def _get_prog():
    global _PROG
    if _PROG is None:
        _PROG = build_program()
    return _PROG


def make_in_maps(inputs):
    x = np.asarray(inputs['x'], np.float32)
    c_w = np.asarray(inputs['c_w'], np.float32)[:, :, 0]
    c_b = np.asarray(inputs['c_b'], np.float32)
    gn_w = np.asarray(inputs['gn_w'], np.float32)
    gn_b = np.asarray(inputs['gn_b'], np.float32)

    perm = []
    for r in range(4):
        perm += list(range(r * 96, (r + 1) * 96))
        perm += list(range(D_MODEL + r * 96, D_MODEL + (r + 1) * 96))
    perm = np.array(perm)
    c_w_p = c_w[perm]
    c_b_p = c_b[perm]

    in_maps = []
    for core in range(8):
        b, rem = divmod(core, 4)
        dirn, nh = divmod(rem, 2)
        rank = rem
        pref = 'f_' if dirn == 0 else 'b_'
        g = lambda k: np.asarray(inputs[pref + k], np.float32)

        x_bc = x[b] if dirn == 0 else np.ascontiguousarray(x[b, :, ::-1])
        in_w = g('in_w')                    # [1536, 384]
        cw = g('conv_w')[:, 0, :]           # [768, 4]
        wtap = np.zeros((128, 6 * D_CONV, 128), np.float32)
        for dt in range(6):
            for tap in range(D_CONV):
                np.fill_diagonal(wtap[:, dt * D_CONV + tap, :],
                                 cw[dt * 128:(dt + 1) * 128, tap])
        xproj_w = g('xproj_w')              # [56, 768]
        rows = np.concatenate([
            xproj_w[:DT_RANK],
            xproj_w[DT_RANK + nh * NH: DT_RANK + (nh + 1) * NH],
            xproj_w[DT_RANK + D_STATE + nh * NH: DT_RANK + D_STATE + (nh + 1) * NH],
        ], 0)                               # [40, 768]
        A = -np.exp(g('A_log'))             # [768, 16]
        Dp = g('D') if nh == 0 else np.zeros(D_INNER, np.float32)
        wdsk = np.zeros((128, 6, 128), np.float32)
        for dt in range(6):
            np.fill_diagonal(wdsk[:, dt, :], Dp[dt * 128:(dt + 1) * 128])
        wc_slice = c_w_p[:, dirn * D_MODEL:(dirn + 1) * D_MODEL]  # [768, 384]

        m = {
            'x_bc': _tiles_pmajor(np.ascontiguousarray(x_bc)).astype(bf),
            'w_zg': _tiles_pmajor(
                np.ascontiguousarray(in_w[D_INNER:].T)).astype(bf),
            'w_xi': _tiles_pmajor(
                np.ascontiguousarray(in_w[:D_INNER].T)).astype(bf),
            'w_tap': wtap.astype(bf),
            'w_xp': _tiles_pmajor(rows.T).astype(bf),
            'w_dt': np.ascontiguousarray(g('dt_w').T).astype(bf),
            'w_out': _tiles_pmajor(g('out_w').T).astype(bf),
            'w_zc': _tiles_pmajor(np.ascontiguousarray(wc_slice.T)).astype(bf),
            'w_dsk': wdsk.astype(bf),
            'conv_b': _vec6(g('conv_b')),
            'dt_b': _vec6(g('dt_b')),
            'a_sc': np.ascontiguousarray(
                A[:, nh * NH:(nh + 1) * NH].reshape(6, 128, NH).transpose(1, 0, 2)),
            'cb_a': np.ascontiguousarray(
                c_b_p[rank * 192: rank * 192 + 96].reshape(96, 1)),
            'cb_b': np.ascontiguousarray(
                c_b_p[rank * 192 + 96:(rank + 1) * 192].reshape(96, 1)),
            'gnw': np.ascontiguousarray(
                gn_w[rank * 96:(rank + 1) * 96].reshape(96, 1)),
            'gnb': np.ascontiguousarray(
                gn_b[rank * 96:(rank + 1) * 96].reshape(96, 1)),
        }
        in_maps.append(m)
    return in_maps


def kernel(**inputs):
    nc = _get_prog()
    in_maps = make_in_maps(inputs)
    res = run_bass_kernel_spmd(nc, in_maps, list(range(8)))
    outs = res.results
    out = np.zeros((B, D_MODEL, L), np.float32)
    for core in range(8):
        b, rank = divmod(core, 4)
        out[b, rank * 96:(rank + 1) * 96, :] = outs[core]['y_out']
    return out


if __name__ == "__main__":
    import reference as ref
    inputs = {k: np.asarray(v) for k, v in ref.setup_inputs().items()}
    got = kernel(**inputs)
    exp = np.asarray(ref.reference(**inputs))
    rel = np.linalg.norm(got - exp) / np.linalg.norm(exp)
    print("rel fro err:", rel)
